# revision 18
# baseline (speedup 1.0000x reference)
"""Causal self-attention (B=2, T=2048, C=1024, H=16) on 8 Trainium2 cores.

Sharding: DP2 over batch x TP4 over heads (4 heads/core). Each core computes
its batch's QKV projection for its heads, RoPE, causal attention, and a
partial c_proj over its 256 input channels. Host sums the 4 partials per
batch and adds b_proj.

v2: all matmul operands in bf16 (same PE rate as fp32r, but FWL fast weight
loads + 2x DVE modes + half the DMA bytes). Weights/x tiles are split
per-128-row slice so the first projection matmul starts as soon as slice 0
lands instead of waiting for the whole tensor. Attention for chunks 0-2 is
woven into the projection stage (chunk tci only needs proj chunks 0..tci);
stage 2 is chunk 3 + all of c_proj, so the PE-hungry c_proj fills the
ScalarE-bound softmax-exp tail. Outputs are written as bf16 partials.

Scores are computed two heads at a time via tile_position row packing, exp
runs on ScalarE straight from PSUM with the 1/sqrt(D) scale fused, and the
causal mask is an affine_select on diagonal blocks only. V carries a ones
column per head so the softmax denominator falls out of the p@v matmul.
"""

import sys

sys.path.insert(0, "/opt/trn_rl_repo")

import math

import ml_dtypes
import numpy as np

import concourse.bass as bass
import concourse.mybir as mybir
import concourse.tile as tile
from concourse import bacc, bass_utils

B, T, C = 2, 2048, 1024
H, D = 16, 64
N_CORES = 8
DP, TP = 2, 4
HPC = H // TP  # heads per core
SC = 512  # t-chunk width / psum bank width
NT = T // SC
NSB = T // 128  # s-blocks

F32 = mybir.dt.float32
BF16 = mybir.dt.bfloat16

_cached = {}


def _build_program():
    nc = bacc.Bacc("TRN2", target_bir_lowering=False, debug=False, num_devices=N_CORES)

    xT_d = nc.dram_tensor("xT", [C, T], BF16, kind="ExternalInput").ap()
    wqk_d = nc.dram_tensor("wqk", [C, 512], BF16, kind="ExternalInput").ap()
    wv_d = nc.dram_tensor("wv", [C, 256], BF16, kind="ExternalInput").ap()
    wpT_d = nc.dram_tensor("wpT", [256, C], BF16, kind="ExternalInput").ap()
    bqk_d = nc.dram_tensor("bqk", [4, 128], F32, kind="ExternalInput").ap()
    bv_d = nc.dram_tensor("bv", [1, 256], F32, kind="ExternalInput").ap()
    cos_d = nc.dram_tensor("cosT", [128, T], BF16, kind="ExternalInput").ap()
    sin_d = nc.dram_tensor("sinT", [128, T], BF16, kind="ExternalInput").ap()
    psw_d = nc.dram_tensor("pswapT", [128, 128], BF16, kind="ExternalInput").ap()
    out_d = nc.dram_tensor("out", [T, C], BF16, kind="ExternalOutput").ap()

    with tile.TileContext(nc) as tc:
        with (
            tc.tile_pool(name="const", bufs=1) as const,
            tc.tile_pool(name="wqkp", bufs=1) as wqkp,
            tc.tile_pool(name="x0p", bufs=1) as x0p,
            tc.tile_pool(name="rotp", bufs=1) as rotp,
            tc.tile_pool(name="vsbp", bufs=1) as vsbp,
            tc.tile_pool(name="ptp", bufs=4) as ptp,
            tc.tile_pool(name="ypairp", bufs=1) as ypairp,
            tc.tile_pool(name="ysbp", bufs=2) as ysbp,
            tc.tile_pool(name="lrowp", bufs=2) as lrowp,
            tc.tile_pool(name="bcp", bufs=2) as bcp,
            tc.tile_pool(name="dumexp", bufs=1) as dumexp,
        ):
            psw_sb = const.tile([128, 128], BF16)
            cos_sb = const.tile([128, T], BF16)
            sin_sb = const.tile([128, T], BF16)
            bqk_sb = const.tile([128, 4], F32)
            bv_row = const.tile([1, 256], F32)
            bv_bc = const.tile([128, 256], F32)
            wpT_sb = const.tile([128, 2, C], BF16)

            # prime the ScalarE exp table set during the initial DMA wait
            dum = dumexp.tile([1, 8], F32)
            nc.vector.memset(dum[:], 0.0)
            nc.scalar.activation(
                out=dum[:], in_=dum[:], func=mybir.ActivationFunctionType.Exp
            )

            wqk_sb = wqkp.tile([128, 8, 512], BF16)
            x0_sb = x0p.tile([128, 8, SC], BF16)
            wqk_r = wqk_d.rearrange("(a b) c -> b a c", b=128)
            wv_r = wv_d.rearrange("(a b) c -> b a c", b=128)
            xT_r = xT_d.rearrange("(a b) c -> b a c", b=128)

            wv_sb = const.tile([128, 8, 256], BF16)

            # DMA issue costs ~650ns of engine time each and the 8 HW rings
            # serialize round-robin, so use FEW, BIG transfers: wqk and x0
            # each as one 1MB DMA on separate engines; tiny consts go via
            # the gpsimd SWDGE queue so they don't delay x0's issue.
            nc.sync.dma_start(out=wqk_sb[:], in_=wqk_r[:, :, :])
            nc.scalar.dma_start(out=x0_sb[:], in_=xT_r[:, :, 0:SC])
            nc.gpsimd.dma_start(out=bqk_sb[:], in_=bqk_d.rearrange("a b -> b a"))
            nc.gpsimd.dma_start(out=bv_row[:], in_=bv_d[:, :])
            nc.gpsimd.dma_start(out=psw_sb[:], in_=psw_d[:, :])
            nc.scalar.dma_start(out=cos_sb[:], in_=cos_d[:, :])
            nc.scalar.dma_start(out=sin_sb[:], in_=sin_d[:, :])
            nc.scalar.dma_start(out=wv_sb[:], in_=wv_r[:, :, :])
            nc.gpsimd.partition_broadcast(bv_bc[:, :], bv_row[0:1, :])
            nc.scalar.dma_start(
                out=wpT_sb[:], in_=wpT_d.rearrange("(a b) c -> b a c", b=128)
            )

            # qT/kT after rope: m=0,1 q head-pairs; m=2,3 k head-pairs
            rot = [
                rotp.tile([128, T], BF16, tag=f"rot{m}", name=f"rot{m}")
                for m in range(4)
            ]
            # v with ones column per head: [128part(t), NSB, HPC*65]
            v_sb = vsbp.tile([128, NSB, HPC * 65], BF16)
            nc.vector.memset(v_sb[:], 1.0)
            ypair = [
                [
                    ypairp.tile(
                        [128, SC], BF16, tag=f"yp{tci}{p}", name=f"yp{tci}{p}"
                    )
                    for p in range(2)
                ]
                for tci in range(NT)
            ]

            def attn_chunk(tci, ps_pool, psy_pool):
                """Scores+softmax+p@v+normalize for one 512-wide t-chunk."""
                t0 = tci * SC
                nsb = tci * 4 + 4
                for p in range(2):
                    psy = [
                        psy_pool.tile([65, SC], F32, tag=f"psy{q}", name=f"psy{q}")
                        for q in range(2)
                    ]
                    for sbi in range(nsb):
                        s0 = sbi * 128
                        ssl = bass.ds(s0, 128)
                        # cols below d0 are causally dead: never computed
                        d0 = max(0, s0 - t0)
                        nn = SC - d0
                        pss = ps_pool.tile([128, 2 * SC], F32, tag="pss", name="pss")
                        nc.tensor.matmul(
                            pss[:, d0:SC],
                            rot[2 + p][0:64, ssl],
                            rot[p][0:64, bass.ds(t0 + d0, nn)],
                            tile_position=(0, 0),
                        )
                        nc.tensor.matmul(
                            pss[:, SC + d0 : 2 * SC],
                            rot[2 + p][64:128, ssl],
                            rot[p][64:128, bass.ds(t0 + d0, nn)],
                            tile_position=(64, 0),
                        )
                        pt = ptp.tile([128, 2 * SC], BF16, tag="pt", name="pt")
                        pt3 = pt[:].rearrange("p (h c) -> p h c", h=2)[:, :, d0:SC]
                        nc.scalar.activation(
                            out=pt3,
                            in_=pss[:].rearrange("p (h c) -> p h c", h=2)[:, :, d0:SC],
                            func=mybir.ActivationFunctionType.Exp,
                            scale=1.0 / math.sqrt(D),
                        )
                        if s0 >= t0:
                            # zero t < s for both heads: keep y' - x >= 0.
                            # only the first 128 cols past the diagonal can
                            # violate causality (x <= 127), so mask just those
                            pt3m = pt3[:, :, 0:128]
                            nc.gpsimd.affine_select(
                                out=pt3m,
                                in_=pt3m,
                                compare_op=mybir.AluOpType.is_ge,
                                fill=0.0,
                                base=0,
                                pattern=[[0, 2], [1, 128]],
                                channel_multiplier=-1,
                            )
                        for q in range(2):
                            h = 2 * p + q
                            nc.tensor.matmul(
                                psy[q][:, d0:SC],
                                v_sb[:, sbi, h * 65 : h * 65 + 65],
                                pt[:, q * SC + d0 : (q + 1) * SC],
                                start=(sbi == 0),
                                stop=(sbi == nsb - 1),
                            )
                    for q in range(2):
                        # free the psum bank right away; l-pipeline from SBUF
                        ysb = ysbp.tile([65, SC], F32, tag="ysb", name="ysb")
                        if q == 0:
                            nc.scalar.copy(ysb[:, :], psy[q][:, :])
                        else:
                            nc.vector.tensor_copy(ysb[:, :], psy[q][:, :])
                        lraw = lrowp.tile([1, SC], F32, tag="lraw", name="lraw")
                        # partition-shifted copy is HW-safe; a partition-
                        # shifted reciprocal is NOT (garbage on HW, fine in sim)
                        nc.vector.tensor_copy(lraw[0:1, :], ysb[64:65, :])
                        lrow0 = lrowp.tile([1, SC], F32, tag="lrow0", name="lrow0")
                        nc.vector.reciprocal_approx_fast(lrow0[0:1, :], lraw[0:1, :])
                        bc = bcp.tile([64, SC], F32, tag="bc", name="bc")
                        nc.gpsimd.partition_broadcast(bc[:, :], lrow0[0:1, :])
                        nc.vector.tensor_mul(
                            ypair[tci][p][q * 64 : (q + 1) * 64, :],
                            ysb[0:64, :],
                            bc[:, :],
                        )

            def cproj_chunk(tci, pso_pool, ostp, qeng):
                t0 = tci * SC
                for ms in range(4):
                    ost = ostp.tile([128, C], BF16, tag="ost", name="ost")
                    for nch2 in range(2):
                        pso = pso_pool.tile([128, 512], F32, tag="pso", name="pso")
                        for kp in range(2):
                            nc.tensor.matmul(
                                pso[:],
                                ypair[tci][kp][:, bass.ts(ms, 128)],
                                wpT_sb[:, kp, bass.ts(nch2, 512)],
                                start=(kp == 0),
                                stop=(kp == 1),
                            )
                        if nch2 == 0:
                            nc.scalar.copy(ost[:, 0:512], pso[:])
                        else:
                            nc.vector.tensor_copy(ost[:, 512:1024], pso[:])
                    qeng[ms % len(qeng)].dma_start(
                        out=out_d[bass.ds(t0 + ms * 128, 128), :], in_=ost[:]
                    )

            # ---- Single schedule scope: projection, attention, and c_proj
            # share pools so the scheduler can weave them with no stage
            # barrier. PSUM: psA(2) + psE(2x2) + psyE(2) = 8 banks.
            with (
                tc.tile_pool(name="xchp", bufs=3) as xchp,
                tc.tile_pool(name="rawp", bufs=5) as rawp,
                tc.tile_pool(name="ttmp", bufs=3) as ttmp,
                tc.tile_pool(name="ostp", bufs=3) as ostp,
                tc.tile_pool(name="psA", bufs=3, space="PSUM") as psA,
                tc.tile_pool(name="psO", bufs=1, space="PSUM") as psO,
                tc.tile_pool(name="psE", bufs=1, space="PSUM") as psE,
                tc.tile_pool(name="psyE", bufs=1, space="PSUM") as psyE,
            ):

                def proj_chunk(nch, rhs_of, vstat_of):
                    sl = bass.ts(nch, SC)
                    # q,k projection: out[m-tile, t-chunk]
                    raw = [
                        rawp.tile([128, SC], BF16, tag="raw", name=f"raw{m}")
                        for m in range(4)
                    ]
                    for m in range(4):
                        ps = psA.tile([128, SC], F32, tag="pa", name="pa")
                        for ct in range(8):
                            nc.tensor.matmul(
                                ps[:],
                                wqk_sb[:, ct, bass.ts(m, 128)],
                                rhs_of(ct),
                                start=(ct == 0),
                                stop=(ct == 7),
                            )
                        nc.scalar.activation(
                            out=raw[m],
                            in_=ps[:],
                            func=mybir.ActivationFunctionType.Identity,
                            bias=bqk_sb[:, m : m + 1],
                        )
                    # v projection for the 4 t-subtiles of this chunk
                    for tml in range(4):
                        tm = nch * 4 + tml
                        psv = psA.tile([128, 256], F32, tag="pa", name="pav")
                        for ct in range(8):
                            nc.tensor.matmul(
                                psv[:],
                                vstat_of(ct, tml),
                                wv_sb[:, ct, :],
                                start=(ct == 0),
                                stop=(ct == 7),
                            )
                        nc.vector.tensor_add(
                            v_sb[:, tm, :]
                            .rearrange("p (h c) -> p h c", h=HPC)[:, :, 0:64],
                            psv[:].rearrange("p (h c) -> p h c", h=HPC),
                            bv_bc[:].rearrange("p (h c) -> p h c", h=HPC),
                        )
                    # rope on the 4 qk tiles for this chunk
                    for m in range(4):
                        psw = psA.tile([128, SC], F32, tag="pa", name="paw")
                        nc.tensor.matmul(psw[:], psw_sb[:], raw[m][:])
                        tmp = ttmp.tile([128, SC], BF16, tag="ttmp")
                        nc.vector.tensor_mul(tmp[:], psw[:], sin_sb[:, sl])
                        nc.vector.tensor_mul(rot[m][:, sl], raw[m][:], cos_sb[:, sl])
                        nc.vector.tensor_add(rot[m][:, sl], rot[m][:, sl], tmp[:])

                xch_next = None
                for nch in range(4):
                    if nch == 0:
                        rhs_of = lambda ct: x0_sb[:, ct, :]
                        vstat_of = lambda ct, tml: x0_sb[:, ct, bass.ts(tml, 128)]
                    else:
                        xch = xch_next
                        rhs_of = lambda ct, xch=xch: xch[:, ct, :]
                        vstat_of = lambda ct, tml, xch=xch: xch[
                            :, ct, bass.ts(tml, 128)
                        ]
                    if nch < 3:
                        xch_next = xchp.tile([128, 8, SC], BF16, tag="xch")
                        qe = nc.sync if nch % 2 == 0 else nc.gpsimd
                        qe.dma_start(
                            out=xch_next[:], in_=xT_r[:, :, bass.ts(nch + 1, SC)]
                        )
                    proj_chunk(nch, rhs_of, vstat_of)
                    # weave in attention as soon as its proj chunk is done
                    # (chunk tci needs proj 0..tci); exp fills ScalarE while
                    # the PE is projection-bound. c_proj of the previous
                    # chunk gives the PE work while exp runs.
                    if nch < 3:
                        attn_chunk(nch, psE, psyE)
                    if nch >= 1:
                        cproj_chunk(nch - 1, psO, ostp, [nc.sync, nc.gpsimd])
                attn_chunk(3, psE, psyE)
                cproj_chunk(3, psO, ostp, [nc.sync, nc.gpsimd])

    nc.compile()
    return nc


def _host_shards(x, w_attn, b_attn, w_proj):
    """Per-core input dicts. Core c: batch c//TP, heads [HPC*(c%TP) .. )."""
    pos = np.arange(T, dtype=np.float64)
    div = np.exp(np.arange(0, D, 2, dtype=np.float64) * (-(math.log(10000.0) / D)))
    sinu = np.outer(pos, div)  # [T, 32]
    bf = ml_dtypes.bfloat16
    cosT = np.tile(np.cos(sinu).T, (4, 1)).astype(bf)  # [128, T]
    sinT = np.tile(np.sin(sinu).T, (4, 1)).astype(bf)

    psw = np.zeros((128, 128), dtype=np.float32)  # P[out,in]
    for blk in (0, 64):
        for j in range(32):
            psw[blk + j, blk + 32 + j] = -1.0
            psw[blk + 32 + j, blk + j] = 1.0
    pswapT = np.ascontiguousarray(psw.T).astype(bf)

    ev = np.arange(0, D, 2)
    od = np.arange(1, D, 2)
    in_maps = []
    for c in range(N_CORES):
        b, lane = c // TP, c % TP
        heads = [HPC * lane + i for i in range(HPC)]
        idx_qk = []
        for off in (0, C):  # q rows then k rows, deinterleaved per head
            for p in range(2):
                for hh in (heads[2 * p], heads[2 * p + 1]):
                    base = off + hh * D
                    idx_qk.extend((base + ev).tolist())
                    idx_qk.extend((base + od).tolist())
        idx_qk = np.array(idx_qk)
        idx_v = np.concatenate([2 * C + h * D + np.arange(D) for h in heads])
        cols_p = np.concatenate([h * D + np.arange(D) for h in heads])
        in_maps.append(
            {
                "xT": np.ascontiguousarray(x[b].T).astype(bf),
                "wqk": np.ascontiguousarray(w_attn[idx_qk, :].T).astype(bf),
                "wv": np.ascontiguousarray(w_attn[idx_v, :].T).astype(bf),
                "wpT": np.ascontiguousarray(w_proj[:, cols_p].T).astype(bf),
                "bqk": np.ascontiguousarray(b_attn[idx_qk].reshape(4, 128)),
                "bv": np.ascontiguousarray(b_attn[idx_v].reshape(1, 256)),
                "cosT": cosT,
                "sinT": sinT,
                "pswapT": pswapT,
            }
        )
    return in_maps


def kernel(x, w_attn, b_attn, w_proj, b_proj, _trace=False):
    x = np.asarray(x, dtype=np.float32)
    w_attn = np.asarray(w_attn, dtype=np.float32)
    b_attn = np.asarray(b_attn, dtype=np.float32)
    w_proj = np.asarray(w_proj, dtype=np.float32)
    b_proj = np.asarray(b_proj, dtype=np.float32)

    if "nc" not in _cached:
        _cached["nc"] = _build_program()
    nc = _cached["nc"]

    in_maps = _host_shards(x, w_attn, b_attn, w_proj)
    res = bass_utils.run_bass_kernel_spmd(
        nc, in_maps, core_ids=list(range(N_CORES)), trace=_trace
    )
    _cached["last_result"] = res

    out = np.empty((B, T, C), dtype=np.float32)
    for b in range(B):
        acc = res.results[b * TP]["out"].astype(np.float32)
        for lane in range(1, TP):
            acc = acc + res.results[b * TP + lane]["out"].astype(np.float32)
        out[b] = acc + b_proj[None, :]
    return out


# revision 21
# speedup vs baseline: 1.3085x; 1.3085x over previous
"""Causal self-attention (B=2, T=2048, C=1024, H=16) on 8 Trainium2 cores.

Sharding: DP2 over batch x TP4 over heads (4 heads/core). Each core computes
its batch's QKV projection for its heads, RoPE, causal attention, and a
partial c_proj over its 256 input channels. Host sums the 4 partials per
batch and adds b_proj.

v2: all matmul operands in bf16 (same PE rate as fp32r, but FWL fast weight
loads + 2x DVE modes + half the DMA bytes). Weights/x tiles are split
per-128-row slice so the first projection matmul starts as soon as slice 0
lands instead of waiting for the whole tensor. Attention for chunks 0-2 is
woven into the projection stage (chunk tci only needs proj chunks 0..tci);
stage 2 is chunk 3 + all of c_proj, so the PE-hungry c_proj fills the
ScalarE-bound softmax-exp tail. Outputs are written as bf16 partials.

Scores are computed two heads at a time via tile_position row packing, exp
runs on ScalarE straight from PSUM with the 1/sqrt(D) scale fused, and the
causal mask is an affine_select on diagonal blocks only. V carries a ones
column per head so the softmax denominator falls out of the p@v matmul.
"""

import sys

sys.path.insert(0, "/opt/trn_rl_repo")

import math

import ml_dtypes
import numpy as np

import concourse.bass as bass
import concourse.mybir as mybir
import concourse.tile as tile
from concourse import bacc, bass_utils

B, T, C = 2, 2048, 1024
H, D = 16, 64
N_CORES = 8
DP, TP = 2, 4
HPC = H // TP  # heads per core
SC = 512  # t-chunk width / psum bank width
NT = T // SC
NSB = T // 128  # s-blocks

F32 = mybir.dt.float32
BF16 = mybir.dt.bfloat16

_cached = {}


def _build_program():
    nc = bacc.Bacc("TRN2", target_bir_lowering=False, debug=False, num_devices=N_CORES)

    xT_d = nc.dram_tensor("xT", [C, T], BF16, kind="ExternalInput").ap()
    wqk_d = nc.dram_tensor("wqk", [C, 512], BF16, kind="ExternalInput").ap()
    wv_d = nc.dram_tensor("wv", [C, 256], BF16, kind="ExternalInput").ap()
    wpT_d = nc.dram_tensor("wpT", [256, C], BF16, kind="ExternalInput").ap()
    bqk_d = nc.dram_tensor("bqk", [4, 128], F32, kind="ExternalInput").ap()
    bv_d = nc.dram_tensor("bv", [1, 256], F32, kind="ExternalInput").ap()
    cos_d = nc.dram_tensor("cosT", [128, T], BF16, kind="ExternalInput").ap()
    sin_d = nc.dram_tensor("sinT", [128, T], BF16, kind="ExternalInput").ap()
    psw_d = nc.dram_tensor("pswapT", [128, 128], BF16, kind="ExternalInput").ap()
    out_d = nc.dram_tensor("out", [T, C], BF16, kind="ExternalOutput").ap()

    with tile.TileContext(nc) as tc:
        with (
            tc.tile_pool(name="const", bufs=1) as const,
            tc.tile_pool(name="wqkp", bufs=1) as wqkp,
            tc.tile_pool(name="x0p", bufs=1) as x0p,
            tc.tile_pool(name="rotp", bufs=1) as rotp,
            tc.tile_pool(name="vsbp", bufs=1) as vsbp,
            tc.tile_pool(name="ptp", bufs=4) as ptp,
            tc.tile_pool(name="ypairp", bufs=1) as ypairp,
            tc.tile_pool(name="ysbp", bufs=2) as ysbp,
            tc.tile_pool(name="lrowp", bufs=2) as lrowp,
            tc.tile_pool(name="bcp", bufs=2) as bcp,
            tc.tile_pool(name="dumexp", bufs=1) as dumexp,
        ):
            psw_sb = const.tile([128, 128], BF16)
            cos_sb = const.tile([128, T], BF16)
            sin_sb = const.tile([128, T], BF16)
            bqk_sb = const.tile([128, 4], F32)
            bv_row = const.tile([1, 256], F32)
            bv_bc = const.tile([128, 256], F32)
            wpT_sb = const.tile([128, 2, C], BF16)

            # prime the ScalarE exp table set during the initial DMA wait
            dum = dumexp.tile([1, 8], F32)
            nc.vector.memset(dum[:], 0.0)
            nc.scalar.activation(
                out=dum[:], in_=dum[:], func=mybir.ActivationFunctionType.Exp
            )

            wqk_sb = wqkp.tile([128, 8, 512], BF16)
            x0_sb = x0p.tile([128, 8, SC], BF16)
            wqk_r = wqk_d.rearrange("(a b) c -> b a c", b=128)
            wv_r = wv_d.rearrange("(a b) c -> b a c", b=128)
            xT_r = xT_d.rearrange("(a b) c -> b a c", b=128)

            wv_sb = const.tile([128, 8, 256], BF16)

            # DMA issue costs ~650ns of engine time each and the 8 HW rings
            # serialize round-robin, so use FEW, BIG transfers: wqk and x0
            # each as one 1MB DMA on separate engines; tiny consts go via
            # the gpsimd SWDGE queue so they don't delay x0's issue.
            nc.sync.dma_start(out=wqk_sb[:], in_=wqk_r[:, :, :])
            nc.scalar.dma_start(out=x0_sb[:], in_=xT_r[:, :, 0:SC])
            nc.gpsimd.dma_start(out=bqk_sb[:], in_=bqk_d.rearrange("a b -> b a"))
            nc.gpsimd.dma_start(out=bv_row[:], in_=bv_d[:, :])
            nc.gpsimd.dma_start(out=psw_sb[:], in_=psw_d[:, :])
            nc.scalar.dma_start(out=cos_sb[:], in_=cos_d[:, :])
            nc.scalar.dma_start(out=sin_sb[:], in_=sin_d[:, :])
            nc.scalar.dma_start(out=wv_sb[:], in_=wv_r[:, :, :])
            nc.gpsimd.partition_broadcast(bv_bc[:, :], bv_row[0:1, :])
            nc.scalar.dma_start(
                out=wpT_sb[:], in_=wpT_d.rearrange("(a b) c -> b a c", b=128)
            )

            # qT/kT after rope: m=0,1 q head-pairs; m=2,3 k head-pairs
            rot = [
                rotp.tile([128, T], BF16, tag=f"rot{m}", name=f"rot{m}")
                for m in range(4)
            ]
            # v with ones column per head: [128part(t), NSB, HPC*65]
            v_sb = vsbp.tile([128, NSB, HPC * 65], BF16)
            nc.vector.memset(v_sb[:], 1.0)
            ypair = [
                [
                    ypairp.tile(
                        [128, SC], BF16, tag=f"yp{tci}{p}", name=f"yp{tci}{p}"
                    )
                    for p in range(2)
                ]
                for tci in range(NT)
            ]

            def attn_chunk(tci, ps_pool, psy_pool):
                """Scores+softmax+p@v+normalize for one 512-wide t-chunk."""
                t0 = tci * SC
                nsb = tci * 4 + 4
                for p in range(2):
                    psy = [
                        psy_pool.tile([65, SC], F32, tag=f"psy{q}", name=f"psy{q}")
                        for q in range(2)
                    ]
                    for sbi in range(nsb):
                        s0 = sbi * 128
                        ssl = bass.ds(s0, 128)
                        # cols below d0 are causally dead: never computed
                        d0 = max(0, s0 - t0)
                        nn = SC - d0
                        pss = ps_pool.tile([128, 2 * SC], F32, tag="pss", name="pss")
                        nc.tensor.matmul(
                            pss[:, d0:SC],
                            rot[2 + p][0:64, ssl],
                            rot[p][0:64, bass.ds(t0 + d0, nn)],
                            tile_position=(0, 0),
                        )
                        nc.tensor.matmul(
                            pss[:, SC + d0 : 2 * SC],
                            rot[2 + p][64:128, ssl],
                            rot[p][64:128, bass.ds(t0 + d0, nn)],
                            tile_position=(64, 0),
                        )
                        pt = ptp.tile([128, 2 * SC], BF16, tag="pt", name="pt")
                        pt3 = pt[:].rearrange("p (h c) -> p h c", h=2)[:, :, d0:SC]
                        nc.scalar.activation(
                            out=pt3,
                            in_=pss[:].rearrange("p (h c) -> p h c", h=2)[:, :, d0:SC],
                            func=mybir.ActivationFunctionType.Exp,
                            scale=1.0 / math.sqrt(D),
                        )
                        if s0 >= t0:
                            # zero t < s for both heads: keep y' - x >= 0.
                            # only the first 128 cols past the diagonal can
                            # violate causality (x <= 127), so mask just those
                            pt3m = pt3[:, :, 0:128]
                            nc.gpsimd.affine_select(
                                out=pt3m,
                                in_=pt3m,
                                compare_op=mybir.AluOpType.is_ge,
                                fill=0.0,
                                base=0,
                                pattern=[[0, 2], [1, 128]],
                                channel_multiplier=-1,
                            )
                        for q in range(2):
                            h = 2 * p + q
                            nc.tensor.matmul(
                                psy[q][:, d0:SC],
                                v_sb[:, sbi, h * 65 : h * 65 + 65],
                                pt[:, q * SC + d0 : (q + 1) * SC],
                                start=(sbi == 0),
                                stop=(sbi == nsb - 1),
                            )
                    for q in range(2):
                        # free the psum bank right away; l-pipeline from SBUF
                        ysb = ysbp.tile([65, SC], F32, tag="ysb", name="ysb")
                        if q == 0:
                            nc.scalar.copy(ysb[:, :], psy[q][:, :])
                        else:
                            nc.vector.tensor_copy(ysb[:, :], psy[q][:, :])
                        lraw = lrowp.tile([1, SC], F32, tag="lraw", name="lraw")
                        # partition-shifted copy is HW-safe; a partition-
                        # shifted reciprocal is NOT (garbage on HW, fine in sim)
                        nc.vector.tensor_copy(lraw[0:1, :], ysb[64:65, :])
                        lrow0 = lrowp.tile([1, SC], F32, tag="lrow0", name="lrow0")
                        nc.vector.reciprocal_approx_fast(lrow0[0:1, :], lraw[0:1, :])
                        bc = bcp.tile([64, SC], F32, tag="bc", name="bc")
                        nc.gpsimd.partition_broadcast(bc[:, :], lrow0[0:1, :])
                        nc.vector.tensor_mul(
                            ypair[tci][p][q * 64 : (q + 1) * 64, :],
                            ysb[0:64, :],
                            bc[:, :],
                        )

            def cproj_chunk(tci, pso_pool, ostp, qeng):
                t0 = tci * SC
                for ms in range(4):
                    ost = ostp.tile([128, C], BF16, tag="ost", name="ost")
                    for nch2 in range(2):
                        pso = pso_pool.tile([128, 512], F32, tag="pa", name="pso")
                        for kp in range(2):
                            nc.tensor.matmul(
                                pso[:],
                                ypair[tci][kp][:, bass.ts(ms, 128)],
                                wpT_sb[:, kp, bass.ts(nch2, 512)],
                                start=(kp == 0),
                                stop=(kp == 1),
                            )
                        if nch2 == 0:
                            nc.scalar.copy(ost[:, 0:512], pso[:])
                        else:
                            nc.vector.tensor_copy(ost[:, 512:1024], pso[:])
                    qeng[ms % len(qeng)].dma_start(
                        out=out_d[bass.ds(t0 + ms * 128, 128), :], in_=ost[:]
                    )

            # ---- Single schedule scope: projection, attention, and c_proj
            # share pools so the scheduler can weave them with no stage
            # barrier. PSUM: psA(2) + psE(2x2) + psyE(2) = 8 banks.
            with (
                tc.tile_pool(name="xchp", bufs=3) as xchp,
                tc.tile_pool(name="rawp", bufs=5) as rawp,
                tc.tile_pool(name="ttmp", bufs=3) as ttmp,
                tc.tile_pool(name="ostp", bufs=3) as ostp,
                tc.tile_pool(name="psA", bufs=2, space="PSUM") as psA,
                tc.tile_pool(name="psE", bufs=2, space="PSUM") as psE,
                tc.tile_pool(name="psyE", bufs=1, space="PSUM") as psyE,
            ):

                def proj_chunk(nch, rhs_of, vstat_of):
                    sl = bass.ts(nch, SC)
                    # q,k projection: out[m-tile, t-chunk]
                    raw = [
                        rawp.tile([128, SC], BF16, tag="raw", name=f"raw{m}")
                        for m in range(4)
                    ]
                    for m in range(4):
                        ps = psA.tile([128, SC], F32, tag="pa", name="pa")
                        for ct in range(8):
                            nc.tensor.matmul(
                                ps[:],
                                wqk_sb[:, ct, bass.ts(m, 128)],
                                rhs_of(ct),
                                start=(ct == 0),
                                stop=(ct == 7),
                            )
                        nc.scalar.activation(
                            out=raw[m],
                            in_=ps[:],
                            func=mybir.ActivationFunctionType.Identity,
                            bias=bqk_sb[:, m : m + 1],
                        )
                    # v projection for the 4 t-subtiles of this chunk
                    for tml in range(4):
                        tm = nch * 4 + tml
                        psv = psA.tile([128, 256], F32, tag="pa", name="pav")
                        for ct in range(8):
                            nc.tensor.matmul(
                                psv[:],
                                vstat_of(ct, tml),
                                wv_sb[:, ct, :],
                                start=(ct == 0),
                                stop=(ct == 7),
                            )
                        nc.vector.tensor_add(
                            v_sb[:, tm, :]
                            .rearrange("p (h c) -> p h c", h=HPC)[:, :, 0:64],
                            psv[:].rearrange("p (h c) -> p h c", h=HPC),
                            bv_bc[:].rearrange("p (h c) -> p h c", h=HPC),
                        )
                    # rope on the 4 qk tiles for this chunk
                    for m in range(4):
                        psw = psA.tile([128, SC], F32, tag="pa", name="paw")
                        nc.tensor.matmul(psw[:], psw_sb[:], raw[m][:])
                        tmp = ttmp.tile([128, SC], BF16, tag="ttmp")
                        nc.vector.tensor_mul(tmp[:], psw[:], sin_sb[:, sl])
                        nc.vector.tensor_mul(rot[m][:, sl], raw[m][:], cos_sb[:, sl])
                        nc.vector.tensor_add(rot[m][:, sl], rot[m][:, sl], tmp[:])

                xch_next = None
                for nch in range(4):
                    if nch == 0:
                        rhs_of = lambda ct: x0_sb[:, ct, :]
                        vstat_of = lambda ct, tml: x0_sb[:, ct, bass.ts(tml, 128)]
                    else:
                        xch = xch_next
                        rhs_of = lambda ct, xch=xch: xch[:, ct, :]
                        vstat_of = lambda ct, tml, xch=xch: xch[
                            :, ct, bass.ts(tml, 128)
                        ]
                    if nch < 3:
                        xch_next = xchp.tile([128, 8, SC], BF16, tag="xch")
                        qe = nc.sync if nch % 2 == 0 else nc.gpsimd
                        qe.dma_start(
                            out=xch_next[:], in_=xT_r[:, :, bass.ts(nch + 1, SC)]
                        )
                    proj_chunk(nch, rhs_of, vstat_of)
                    # weave in attention as soon as its proj chunk is done
                    # (chunk tci needs proj 0..tci); exp fills ScalarE while
                    # the PE is projection-bound. c_proj of the previous
                    # chunk gives the PE work while exp runs.
                    if nch < 3:
                        attn_chunk(nch, psE, psyE)
                    if nch >= 1:
                        cproj_chunk(nch - 1, psA, ostp, [nc.sync, nc.gpsimd])
                attn_chunk(3, psE, psyE)
                cproj_chunk(3, psA, ostp, [nc.sync, nc.gpsimd])

    nc.compile()
    return nc


def _host_shards(x, w_attn, b_attn, w_proj):
    """Per-core input dicts. Core c: batch c//TP, heads [HPC*(c%TP) .. )."""
    pos = np.arange(T, dtype=np.float64)
    div = np.exp(np.arange(0, D, 2, dtype=np.float64) * (-(math.log(10000.0) / D)))
    sinu = np.outer(pos, div)  # [T, 32]
    bf = ml_dtypes.bfloat16
    cosT = np.tile(np.cos(sinu).T, (4, 1)).astype(bf)  # [128, T]
    sinT = np.tile(np.sin(sinu).T, (4, 1)).astype(bf)

    psw = np.zeros((128, 128), dtype=np.float32)  # P[out,in]
    for blk in (0, 64):
        for j in range(32):
            psw[blk + j, blk + 32 + j] = -1.0
            psw[blk + 32 + j, blk + j] = 1.0
    pswapT = np.ascontiguousarray(psw.T).astype(bf)

    ev = np.arange(0, D, 2)
    od = np.arange(1, D, 2)
    in_maps = []
    for c in range(N_CORES):
        b, lane = c // TP, c % TP
        heads = [HPC * lane + i for i in range(HPC)]
        idx_qk = []
        for off in (0, C):  # q rows then k rows, deinterleaved per head
            for p in range(2):
                for hh in (heads[2 * p], heads[2 * p + 1]):
                    base = off + hh * D
                    idx_qk.extend((base + ev).tolist())
                    idx_qk.extend((base + od).tolist())
        idx_qk = np.array(idx_qk)
        idx_v = np.concatenate([2 * C + h * D + np.arange(D) for h in heads])
        cols_p = np.concatenate([h * D + np.arange(D) for h in heads])
        in_maps.append(
            {
                "xT": np.ascontiguousarray(x[b].T).astype(bf),
                "wqk": np.ascontiguousarray(w_attn[idx_qk, :].T).astype(bf),
                "wv": np.ascontiguousarray(w_attn[idx_v, :].T).astype(bf),
                "wpT": np.ascontiguousarray(w_proj[:, cols_p].T).astype(bf),
                "bqk": np.ascontiguousarray(b_attn[idx_qk].reshape(4, 128)),
                "bv": np.ascontiguousarray(b_attn[idx_v].reshape(1, 256)),
                "cosT": cosT,
                "sinT": sinT,
                "pswapT": pswapT,
            }
        )
    return in_maps


def kernel(x, w_attn, b_attn, w_proj, b_proj, _trace=False):
    x = np.asarray(x, dtype=np.float32)
    w_attn = np.asarray(w_attn, dtype=np.float32)
    b_attn = np.asarray(b_attn, dtype=np.float32)
    w_proj = np.asarray(w_proj, dtype=np.float32)
    b_proj = np.asarray(b_proj, dtype=np.float32)

    if "nc" not in _cached:
        _cached["nc"] = _build_program()
    nc = _cached["nc"]

    in_maps = _host_shards(x, w_attn, b_attn, w_proj)
    res = bass_utils.run_bass_kernel_spmd(
        nc, in_maps, core_ids=list(range(N_CORES)), trace=_trace
    )
    _cached["last_result"] = res

    out = np.empty((B, T, C), dtype=np.float32)
    for b in range(B):
        acc = res.results[b * TP]["out"].astype(np.float32)
        for lane in range(1, TP):
            acc = acc + res.results[b * TP + lane]["out"].astype(np.float32)
        out[b] = acc + b_proj[None, :]
    return out


# revision 24
# speedup vs baseline: 1.3175x; 1.0069x over previous
"""Causal self-attention (B=2, T=2048, C=1024, H=16) on 8 Trainium2 cores.

Sharding: DP2 over batch x TP4 over heads (4 heads/core). Each core computes
its batch's QKV projection for its heads, RoPE, causal attention, and a
partial c_proj over its 256 input channels. Host sums the 4 partials per
batch and adds b_proj.

v2: all matmul operands in bf16 (same PE rate as fp32r, but FWL fast weight
loads + 2x DVE modes + half the DMA bytes). Weights/x tiles are split
per-128-row slice so the first projection matmul starts as soon as slice 0
lands instead of waiting for the whole tensor. Attention for chunks 0-2 is
woven into the projection stage (chunk tci only needs proj chunks 0..tci);
stage 2 is chunk 3 + all of c_proj, so the PE-hungry c_proj fills the
ScalarE-bound softmax-exp tail. Outputs are written as bf16 partials.

Scores are computed two heads at a time via tile_position row packing, exp
runs on ScalarE straight from PSUM with the 1/sqrt(D) scale fused, and the
causal mask is an affine_select on diagonal blocks only. V carries a ones
column per head so the softmax denominator falls out of the p@v matmul.
"""

import sys

sys.path.insert(0, "/opt/trn_rl_repo")

import math

import ml_dtypes
import numpy as np

import concourse.bass as bass
import concourse.mybir as mybir
import concourse.tile as tile
from concourse import bacc, bass_utils

B, T, C = 2, 2048, 1024
H, D = 16, 64
N_CORES = 8
DP, TP = 2, 4
HPC = H // TP  # heads per core
SC = 512  # t-chunk width / psum bank width
NT = T // SC
NSB = T // 128  # s-blocks

F32 = mybir.dt.float32
BF16 = mybir.dt.bfloat16

_cached = {}


def _build_program():
    nc = bacc.Bacc("TRN2", target_bir_lowering=False, debug=False, num_devices=N_CORES)

    xT_d = nc.dram_tensor("xT", [C, T], BF16, kind="ExternalInput").ap()
    wqk_d = nc.dram_tensor("wqk", [C, 512], BF16, kind="ExternalInput").ap()
    wv_d = nc.dram_tensor("wv", [C, 256], BF16, kind="ExternalInput").ap()
    wpT_d = nc.dram_tensor("wpT", [256, C], BF16, kind="ExternalInput").ap()
    bqk_d = nc.dram_tensor("bqk", [4, 128], F32, kind="ExternalInput").ap()
    bv_d = nc.dram_tensor("bv", [1, 256], F32, kind="ExternalInput").ap()
    cos_d = nc.dram_tensor("cosT", [128, T], BF16, kind="ExternalInput").ap()
    sin_d = nc.dram_tensor("sinT", [128, T], BF16, kind="ExternalInput").ap()
    psw_d = nc.dram_tensor("pswapT", [128, 128], BF16, kind="ExternalInput").ap()
    out_d = nc.dram_tensor("out", [T, C], BF16, kind="ExternalOutput").ap()

    with tile.TileContext(nc) as tc:
        with (
            tc.tile_pool(name="const", bufs=1) as const,
            tc.tile_pool(name="wqkp", bufs=1) as wqkp,
            tc.tile_pool(name="x0p", bufs=1) as x0p,
            tc.tile_pool(name="rotp", bufs=1) as rotp,
            tc.tile_pool(name="vsbp", bufs=1) as vsbp,
            tc.tile_pool(name="ptp", bufs=4) as ptp,
            tc.tile_pool(name="ypairp", bufs=1) as ypairp,
            tc.tile_pool(name="ysbp", bufs=2) as ysbp,
            tc.tile_pool(name="lrowp", bufs=2) as lrowp,
            tc.tile_pool(name="bcp", bufs=2) as bcp,
            tc.tile_pool(name="dumexp", bufs=1) as dumexp,
        ):
            psw_sb = const.tile([128, 128], BF16)
            cos_sb = const.tile([128, T], BF16)
            sin_sb = const.tile([128, T], BF16)
            bqk_sb = const.tile([128, 4], F32)
            bv_row = const.tile([1, 256], F32)
            bv_bc = const.tile([128, 256], F32)
            wpT_sb = const.tile([128, 2, C], BF16)

            # prime the ScalarE exp table set during the initial DMA wait
            dum = dumexp.tile([1, 8], F32)
            nc.vector.memset(dum[:], 0.0)
            nc.scalar.activation(
                out=dum[:], in_=dum[:], func=mybir.ActivationFunctionType.Exp
            )

            wqk_sb = wqkp.tile([128, 8, 512], BF16)
            x0_sb = x0p.tile([128, 8, SC], BF16)
            wqk_r = wqk_d.rearrange("(a b) c -> b a c", b=128)
            wv_r = wv_d.rearrange("(a b) c -> b a c", b=128)
            xT_r = xT_d.rearrange("(a b) c -> b a c", b=128)

            wv_sb = const.tile([128, 8, 256], BF16)

            # DMA issue costs ~650ns of engine time each and the 8 HW rings
            # serialize round-robin, so use FEW, BIG transfers: wqk and x0
            # each as one 1MB DMA on separate engines; tiny consts go via
            # the gpsimd SWDGE queue so they don't delay x0's issue.
            nc.sync.dma_start(out=wqk_sb[:], in_=wqk_r[:, :, :])
            nc.scalar.dma_start(out=x0_sb[:], in_=xT_r[:, :, 0:SC])
            nc.gpsimd.dma_start(out=bqk_sb[:], in_=bqk_d.rearrange("a b -> b a"))
            nc.gpsimd.dma_start(out=bv_row[:], in_=bv_d[:, :])
            nc.gpsimd.dma_start(out=psw_sb[:], in_=psw_d[:, :])
            nc.scalar.dma_start(out=cos_sb[:], in_=cos_d[:, :])
            nc.scalar.dma_start(out=sin_sb[:], in_=sin_d[:, :])
            nc.scalar.dma_start(out=wv_sb[:], in_=wv_r[:, :, :])
            nc.gpsimd.partition_broadcast(bv_bc[:, :], bv_row[0:1, :])

            # qT/kT after rope: m=0,1 q head-pairs; m=2,3 k head-pairs
            rot = [
                rotp.tile([128, T], BF16, tag=f"rot{m}", name=f"rot{m}")
                for m in range(4)
            ]
            # v with ones column per head: [128part(t), NSB, HPC*65]
            v_sb = vsbp.tile([128, NSB, HPC * 65], BF16)
            nc.vector.memset(v_sb[:], 1.0)
            ypair = [
                [
                    ypairp.tile(
                        [128, SC], BF16, tag=f"yp{tci}{p}", name=f"yp{tci}{p}"
                    )
                    for p in range(2)
                ]
                for tci in range(NT)
            ]

            def attn_chunk(tci, ps_pool, psy_pool):
                """Scores+softmax+p@v+normalize for one 512-wide t-chunk."""
                t0 = tci * SC
                nsb = tci * 4 + 4
                for p in range(2):
                    psy = [
                        psy_pool.tile([65, SC], F32, tag=f"psy{q}", name=f"psy{q}")
                        for q in range(2)
                    ]
                    for sbi in range(nsb):
                        s0 = sbi * 128
                        ssl = bass.ds(s0, 128)
                        # cols below d0 are causally dead: never computed
                        d0 = max(0, s0 - t0)
                        nn = SC - d0
                        pss = ps_pool.tile([128, 2 * SC], F32, tag="pss", name="pss")
                        nc.tensor.matmul(
                            pss[:, d0:SC],
                            rot[2 + p][0:64, ssl],
                            rot[p][0:64, bass.ds(t0 + d0, nn)],
                            tile_position=(0, 0),
                        )
                        nc.tensor.matmul(
                            pss[:, SC + d0 : 2 * SC],
                            rot[2 + p][64:128, ssl],
                            rot[p][64:128, bass.ds(t0 + d0, nn)],
                            tile_position=(64, 0),
                        )
                        pt = ptp.tile([128, 2 * SC], BF16, tag="pt", name="pt")
                        pt3 = pt[:].rearrange("p (h c) -> p h c", h=2)[:, :, d0:SC]
                        nc.scalar.activation(
                            out=pt3,
                            in_=pss[:].rearrange("p (h c) -> p h c", h=2)[:, :, d0:SC],
                            func=mybir.ActivationFunctionType.Exp,
                            scale=1.0 / math.sqrt(D),
                        )
                        if s0 >= t0:
                            # zero t < s for both heads: keep y' - x >= 0.
                            # only the first 128 cols past the diagonal can
                            # violate causality (x <= 127), so mask just those
                            pt3m = pt3[:, :, 0:128]
                            nc.gpsimd.affine_select(
                                out=pt3m,
                                in_=pt3m,
                                compare_op=mybir.AluOpType.is_ge,
                                fill=0.0,
                                base=0,
                                pattern=[[0, 2], [1, 128]],
                                channel_multiplier=-1,
                            )
                        for q in range(2):
                            h = 2 * p + q
                            nc.tensor.matmul(
                                psy[q][:, d0:SC],
                                v_sb[:, sbi, h * 65 : h * 65 + 65],
                                pt[:, q * SC + d0 : (q + 1) * SC],
                                start=(sbi == 0),
                                stop=(sbi == nsb - 1),
                            )
                    for q in range(2):
                        # free the psum bank right away; l-pipeline from SBUF
                        ysb = ysbp.tile([65, SC], F32, tag="ysb", name="ysb")
                        if q == 0:
                            nc.scalar.copy(ysb[:, :], psy[q][:, :])
                        else:
                            nc.vector.tensor_copy(ysb[:, :], psy[q][:, :])
                        lraw = lrowp.tile([1, SC], F32, tag="lraw", name="lraw")
                        # partition-shifted copy is HW-safe; a partition-
                        # shifted reciprocal is NOT (garbage on HW, fine in sim)
                        nc.vector.tensor_copy(lraw[0:1, :], ysb[64:65, :])
                        lrow0 = lrowp.tile([1, SC], F32, tag="lrow0", name="lrow0")
                        nc.vector.reciprocal_approx_fast(lrow0[0:1, :], lraw[0:1, :])
                        bc = bcp.tile([64, SC], F32, tag="bc", name="bc")
                        nc.gpsimd.partition_broadcast(bc[:, :], lrow0[0:1, :])
                        nc.vector.tensor_mul(
                            ypair[tci][p][q * 64 : (q + 1) * 64, :],
                            ysb[0:64, :],
                            bc[:, :],
                        )

            def cproj_chunk(tci, pso_pool, ostp, qeng):
                t0 = tci * SC
                for ms in range(4):
                    ost = ostp.tile([128, C], BF16, tag="ost", name="ost")
                    for nch2 in range(2):
                        pso = pso_pool.tile([128, 512], F32, tag="pa", name="pso")
                        for kp in range(2):
                            nc.tensor.matmul(
                                pso[:],
                                ypair[tci][kp][:, bass.ts(ms, 128)],
                                wpT_sb[:, kp, bass.ts(nch2, 512)],
                                start=(kp == 0),
                                stop=(kp == 1),
                            )
                        if nch2 == 0:
                            nc.scalar.copy(ost[:, 0:512], pso[:])
                        else:
                            nc.vector.tensor_copy(ost[:, 512:1024], pso[:])
                    qeng[ms % len(qeng)].dma_start(
                        out=out_d[bass.ds(t0 + ms * 128, 128), :], in_=ost[:]
                    )

            # ---- Single schedule scope: projection, attention, and c_proj
            # share pools so the scheduler can weave them with no stage
            # barrier. PSUM: psA(2) + psE(2x2) + psyE(2) = 8 banks.
            with (
                tc.tile_pool(name="xchp", bufs=3) as xchp,
                tc.tile_pool(name="rawp", bufs=5) as rawp,
                tc.tile_pool(name="ttmp", bufs=3) as ttmp,
                tc.tile_pool(name="ostp", bufs=3) as ostp,
                tc.tile_pool(name="psA", bufs=2, space="PSUM") as psA,
                tc.tile_pool(name="psE", bufs=2, space="PSUM") as psE,
                tc.tile_pool(name="psyE", bufs=1, space="PSUM") as psyE,
            ):

                def proj_chunk(nch, rhs_of, vstat_of):
                    sl = bass.ts(nch, SC)
                    # q,k projection: out[m-tile, t-chunk]
                    raw = [
                        rawp.tile([128, SC], BF16, tag="raw", name=f"raw{m}")
                        for m in range(4)
                    ]
                    for m in range(4):
                        ps = psA.tile([128, SC], F32, tag="pa", name="pa")
                        for ct in range(8):
                            nc.tensor.matmul(
                                ps[:],
                                wqk_sb[:, ct, bass.ts(m, 128)],
                                rhs_of(ct),
                                start=(ct == 0),
                                stop=(ct == 7),
                            )
                        nc.scalar.activation(
                            out=raw[m],
                            in_=ps[:],
                            func=mybir.ActivationFunctionType.Identity,
                            bias=bqk_sb[:, m : m + 1],
                        )
                    # v projection for the 4 t-subtiles of this chunk
                    for tml in range(4):
                        tm = nch * 4 + tml
                        psv = psA.tile([128, 256], F32, tag="pa", name="pav")
                        for ct in range(8):
                            nc.tensor.matmul(
                                psv[:],
                                vstat_of(ct, tml),
                                wv_sb[:, ct, :],
                                start=(ct == 0),
                                stop=(ct == 7),
                            )
                        nc.vector.tensor_add(
                            v_sb[:, tm, :]
                            .rearrange("p (h c) -> p h c", h=HPC)[:, :, 0:64],
                            psv[:].rearrange("p (h c) -> p h c", h=HPC),
                            bv_bc[:].rearrange("p (h c) -> p h c", h=HPC),
                        )
                    # rope on the 4 qk tiles for this chunk
                    for m in range(4):
                        psw = psA.tile([128, SC], F32, tag="pa", name="paw")
                        nc.tensor.matmul(psw[:], psw_sb[:], raw[m][:])
                        tmp = ttmp.tile([128, SC], BF16, tag="ttmp")
                        nc.vector.tensor_mul(tmp[:], psw[:], sin_sb[:, sl])
                        nc.vector.tensor_mul(rot[m][:, sl], raw[m][:], cos_sb[:, sl])
                        nc.vector.tensor_add(rot[m][:, sl], rot[m][:, sl], tmp[:])

                # xch1 races the startup window; xch2/3 and wpT are issued
                # from the gpsimd engine stream mid-loop so their transfers
                # don't steal HBM bandwidth from the startup-critical loads.
                xchs = {1: xchp.tile([128, 8, SC], BF16, tag="xch", name="xch1")}
                nc.sync.dma_start(out=xchs[1][:], in_=xT_r[:, :, bass.ts(1, SC)])
                for nch in range(4):
                    if nch == 0:
                        rhs_of = lambda ct: x0_sb[:, ct, :]
                        vstat_of = lambda ct, tml: x0_sb[:, ct, bass.ts(tml, 128)]
                    else:
                        xch = xchs[nch]
                        rhs_of = lambda ct, xch=xch: xch[:, ct, :]
                        vstat_of = lambda ct, tml, xch=xch: xch[
                            :, ct, bass.ts(tml, 128)
                        ]
                    proj_chunk(nch, rhs_of, vstat_of)
                    # weave in attention as soon as its proj chunk is done
                    # (chunk tci needs proj 0..tci); exp fills ScalarE while
                    # the PE is projection-bound. c_proj of the previous
                    # chunk gives the PE work while exp runs.
                    if nch < 3:
                        attn_chunk(nch, psE, psyE)
                    if nch == 0:
                        nc.gpsimd.dma_start(
                            out=wpT_sb[:],
                            in_=wpT_d.rearrange("(a b) c -> b a c", b=128),
                        )
                    if nch + 2 <= 3:
                        xchs[nch + 2] = xchp.tile(
                            [128, 8, SC], BF16, tag="xch", name=f"xch{nch + 2}"
                        )
                        nc.gpsimd.dma_start(
                            out=xchs[nch + 2][:], in_=xT_r[:, :, bass.ts(nch + 2, SC)]
                        )
                    if nch >= 1:
                        cproj_chunk(nch - 1, psA, ostp, [nc.sync])
                attn_chunk(3, psE, psyE)
                cproj_chunk(3, psA, ostp, [nc.sync])

    nc.compile()
    return nc


def _host_shards(x, w_attn, b_attn, w_proj):
    """Per-core input dicts. Core c: batch c//TP, heads [HPC*(c%TP) .. )."""
    pos = np.arange(T, dtype=np.float64)
    div = np.exp(np.arange(0, D, 2, dtype=np.float64) * (-(math.log(10000.0) / D)))
    sinu = np.outer(pos, div)  # [T, 32]
    bf = ml_dtypes.bfloat16
    cosT = np.tile(np.cos(sinu).T, (4, 1)).astype(bf)  # [128, T]
    sinT = np.tile(np.sin(sinu).T, (4, 1)).astype(bf)

    psw = np.zeros((128, 128), dtype=np.float32)  # P[out,in]
    for blk in (0, 64):
        for j in range(32):
            psw[blk + j, blk + 32 + j] = -1.0
            psw[blk + 32 + j, blk + j] = 1.0
    pswapT = np.ascontiguousarray(psw.T).astype(bf)

    ev = np.arange(0, D, 2)
    od = np.arange(1, D, 2)
    in_maps = []
    for c in range(N_CORES):
        b, lane = c // TP, c % TP
        heads = [HPC * lane + i for i in range(HPC)]
        idx_qk = []
        for off in (0, C):  # q rows then k rows, deinterleaved per head
            for p in range(2):
                for hh in (heads[2 * p], heads[2 * p + 1]):
                    base = off + hh * D
                    idx_qk.extend((base + ev).tolist())
                    idx_qk.extend((base + od).tolist())
        idx_qk = np.array(idx_qk)
        idx_v = np.concatenate([2 * C + h * D + np.arange(D) for h in heads])
        cols_p = np.concatenate([h * D + np.arange(D) for h in heads])
        in_maps.append(
            {
                "xT": np.ascontiguousarray(x[b].T).astype(bf),
                "wqk": np.ascontiguousarray(w_attn[idx_qk, :].T).astype(bf),
                "wv": np.ascontiguousarray(w_attn[idx_v, :].T).astype(bf),
                "wpT": np.ascontiguousarray(w_proj[:, cols_p].T).astype(bf),
                "bqk": np.ascontiguousarray(b_attn[idx_qk].reshape(4, 128)),
                "bv": np.ascontiguousarray(b_attn[idx_v].reshape(1, 256)),
                "cosT": cosT,
                "sinT": sinT,
                "pswapT": pswapT,
            }
        )
    return in_maps


def kernel(x, w_attn, b_attn, w_proj, b_proj, _trace=False):
    x = np.asarray(x, dtype=np.float32)
    w_attn = np.asarray(w_attn, dtype=np.float32)
    b_attn = np.asarray(b_attn, dtype=np.float32)
    w_proj = np.asarray(w_proj, dtype=np.float32)
    b_proj = np.asarray(b_proj, dtype=np.float32)

    if "nc" not in _cached:
        _cached["nc"] = _build_program()
    nc = _cached["nc"]

    in_maps = _host_shards(x, w_attn, b_attn, w_proj)
    res = bass_utils.run_bass_kernel_spmd(
        nc, in_maps, core_ids=list(range(N_CORES)), trace=_trace
    )
    _cached["last_result"] = res

    out = np.empty((B, T, C), dtype=np.float32)
    for b in range(B):
        acc = res.results[b * TP]["out"].astype(np.float32)
        for lane in range(1, TP):
            acc = acc + res.results[b * TP + lane]["out"].astype(np.float32)
        out[b] = acc + b_proj[None, :]
    return out


# revision 26
# speedup vs baseline: 1.3772x; 1.0453x over previous
"""Causal self-attention (B=2, T=2048, C=1024, H=16) on 8 Trainium2 cores.

Sharding: DP2 over batch x TP4 over heads (4 heads/core). Each core computes
its batch's QKV projection for its heads, RoPE, causal attention, and a
partial c_proj over its 256 input channels. Host sums the 4 partials per
batch and adds b_proj.

v2: all matmul operands in bf16 (same PE rate as fp32r, but FWL fast weight
loads + 2x DVE modes + half the DMA bytes). Weights/x tiles are split
per-128-row slice so the first projection matmul starts as soon as slice 0
lands instead of waiting for the whole tensor. Attention for chunks 0-2 is
woven into the projection stage (chunk tci only needs proj chunks 0..tci);
stage 2 is chunk 3 + all of c_proj, so the PE-hungry c_proj fills the
ScalarE-bound softmax-exp tail. Outputs are written as bf16 partials.

Scores are computed two heads at a time via tile_position row packing, exp
runs on ScalarE straight from PSUM with the 1/sqrt(D) scale fused, and the
causal mask is an affine_select on diagonal blocks only. V carries a ones
column per head so the softmax denominator falls out of the p@v matmul.
"""

import sys

sys.path.insert(0, "/opt/trn_rl_repo")

import math

import ml_dtypes
import numpy as np

import concourse.bass as bass
import concourse.mybir as mybir
import concourse.tile as tile
from concourse import bacc, bass_utils

B, T, C = 2, 2048, 1024
H, D = 16, 64
N_CORES = 8
DP, TP = 2, 4
HPC = H // TP  # heads per core
SC = 512  # t-chunk width / psum bank width
NT = T // SC
NSB = T // 128  # s-blocks

F32 = mybir.dt.float32
BF16 = mybir.dt.bfloat16

_cached = {}


def _build_program():
    nc = bacc.Bacc("TRN2", target_bir_lowering=False, debug=False, num_devices=N_CORES)

    xT_d = nc.dram_tensor("xT", [C, T], BF16, kind="ExternalInput").ap()
    wqk_d = nc.dram_tensor("wqk", [C, 512], BF16, kind="ExternalInput").ap()
    wv_d = nc.dram_tensor("wv", [C, 256], BF16, kind="ExternalInput").ap()
    wpT_d = nc.dram_tensor("wpT", [256, C], BF16, kind="ExternalInput").ap()
    bqk_d = nc.dram_tensor("bqk", [4, 128], F32, kind="ExternalInput").ap()
    bv_d = nc.dram_tensor("bv", [1, 256], F32, kind="ExternalInput").ap()
    cos_d = nc.dram_tensor("cosT", [128, T], BF16, kind="ExternalInput").ap()
    sin_d = nc.dram_tensor("sinT", [128, T], BF16, kind="ExternalInput").ap()
    psw_d = nc.dram_tensor("pswapT", [128, 128], BF16, kind="ExternalInput").ap()
    out_d = nc.dram_tensor("out", [T, C], BF16, kind="ExternalOutput").ap()

    with tile.TileContext(nc) as tc:
        with (
            tc.tile_pool(name="const", bufs=1) as const,
            tc.tile_pool(name="wqkp", bufs=1) as wqkp,
            tc.tile_pool(name="x0p", bufs=1) as x0p,
            tc.tile_pool(name="rotp", bufs=1) as rotp,
            tc.tile_pool(name="vsbp", bufs=1) as vsbp,
            tc.tile_pool(name="ptp", bufs=4) as ptp,
            tc.tile_pool(name="ypairp", bufs=1) as ypairp,
            tc.tile_pool(name="ysbp", bufs=2) as ysbp,
            tc.tile_pool(name="lrowp", bufs=2) as lrowp,
            tc.tile_pool(name="bcp", bufs=2) as bcp,
            tc.tile_pool(name="dumexp", bufs=1) as dumexp,
        ):
            psw_sb = const.tile([128, 128], BF16)
            cos_sb = const.tile([128, T], BF16)
            sin_sb = const.tile([128, T], BF16)
            bqk_sb = const.tile([128, 4], F32)
            bv_row = const.tile([1, 256], F32)
            bv_bc = const.tile([128, 256], F32)
            wpT_sb = const.tile([128, 2, C], BF16)

            # prime the ScalarE exp table set during the initial DMA wait
            dum = dumexp.tile([1, 8], F32)
            nc.vector.memset(dum[:], 0.0)
            nc.scalar.activation(
                out=dum[:], in_=dum[:], func=mybir.ActivationFunctionType.Exp
            )

            wqk_sb = wqkp.tile([128, 8, 512], BF16)
            x0_sb = x0p.tile([128, 8, SC], BF16)
            wqk_r = wqk_d.rearrange("(a b) c -> b a c", b=128)
            wv_r = wv_d.rearrange("(a b) c -> b a c", b=128)
            xT_r = xT_d.rearrange("(a b) c -> b a c", b=128)

            wv_sb = const.tile([128, 8, 256], BF16)

            # DMA issue costs ~650ns of engine time each and the 8 HW rings
            # serialize round-robin, so use FEW, BIG transfers: wqk and x0
            # each as one 1MB DMA on separate engines; tiny consts go via
            # the gpsimd SWDGE queue so they don't delay x0's issue.
            # startup-critical loads only: first qk accumulation needs the
            # m=0 wqk slice + all of x0. Everything else is either tiny or
            # gated into the gpsimd stream below so the HBM rings don't
            # fair-share away the critical path's bandwidth.
            nc.sync.dma_start(out=wqk_sb[:, :, 0:128], in_=wqk_r[:, :, 0:128])
            nc.scalar.dma_start(out=x0_sb[:], in_=xT_r[:, :, 0:SC])
            nc.sync.dma_start(out=wqk_sb[:, :, 128:512], in_=wqk_r[:, :, 128:512])
            nc.gpsimd.dma_start(out=bqk_sb[:], in_=bqk_d.rearrange("a b -> b a"))
            nc.gpsimd.dma_start(out=bv_row[:], in_=bv_d[:, :])
            nc.gpsimd.dma_start(out=psw_sb[:], in_=psw_d[:, :])
            nc.scalar.dma_start(out=cos_sb[:, 0:1024], in_=cos_d[:, 0:1024])
            nc.scalar.dma_start(out=sin_sb[:, 0:1024], in_=sin_d[:, 0:1024])
            nc.sync.dma_start(out=wv_sb[:], in_=wv_r[:, :, :])
            nc.gpsimd.partition_broadcast(bv_bc[:, :], bv_row[0:1, :])

            # qT/kT after rope: m=0,1 q head-pairs; m=2,3 k head-pairs
            rot = [
                rotp.tile([128, T], BF16, tag=f"rot{m}", name=f"rot{m}")
                for m in range(4)
            ]
            # v with ones column per head: [128part(t), NSB, HPC*65]
            v_sb = vsbp.tile([128, NSB, HPC * 65], BF16)
            nc.vector.memset(v_sb[:], 1.0)
            ypair = [
                [
                    ypairp.tile(
                        [128, SC], BF16, tag=f"yp{tci}{p}", name=f"yp{tci}{p}"
                    )
                    for p in range(2)
                ]
                for tci in range(NT)
            ]

            def attn_chunk(tci, ps_pool, psy_pool):
                """Scores+softmax+p@v+normalize for one 512-wide t-chunk."""
                t0 = tci * SC
                nsb = tci * 4 + 4
                for p in range(2):
                    psy = [
                        psy_pool.tile([65, SC], F32, tag=f"psy{q}", name=f"psy{q}")
                        for q in range(2)
                    ]
                    for sbi in range(nsb):
                        s0 = sbi * 128
                        ssl = bass.ds(s0, 128)
                        # cols below d0 are causally dead: never computed
                        d0 = max(0, s0 - t0)
                        nn = SC - d0
                        pss = ps_pool.tile([128, 2 * SC], F32, tag="pss", name="pss")
                        nc.tensor.matmul(
                            pss[:, d0:SC],
                            rot[2 + p][0:64, ssl],
                            rot[p][0:64, bass.ds(t0 + d0, nn)],
                            tile_position=(0, 0),
                        )
                        nc.tensor.matmul(
                            pss[:, SC + d0 : 2 * SC],
                            rot[2 + p][64:128, ssl],
                            rot[p][64:128, bass.ds(t0 + d0, nn)],
                            tile_position=(64, 0),
                        )
                        pt = ptp.tile([128, 2 * SC], BF16, tag="pt", name="pt")
                        pt3 = pt[:].rearrange("p (h c) -> p h c", h=2)[:, :, d0:SC]
                        nc.scalar.activation(
                            out=pt3,
                            in_=pss[:].rearrange("p (h c) -> p h c", h=2)[:, :, d0:SC],
                            func=mybir.ActivationFunctionType.Exp,
                            scale=1.0 / math.sqrt(D),
                        )
                        if s0 >= t0:
                            # zero t < s for both heads: keep y' - x >= 0.
                            # only the first 128 cols past the diagonal can
                            # violate causality (x <= 127), so mask just those
                            pt3m = pt3[:, :, 0:128]
                            nc.gpsimd.affine_select(
                                out=pt3m,
                                in_=pt3m,
                                compare_op=mybir.AluOpType.is_ge,
                                fill=0.0,
                                base=0,
                                pattern=[[0, 2], [1, 128]],
                                channel_multiplier=-1,
                            )
                        for q in range(2):
                            h = 2 * p + q
                            nc.tensor.matmul(
                                psy[q][:, d0:SC],
                                v_sb[:, sbi, h * 65 : h * 65 + 65],
                                pt[:, q * SC + d0 : (q + 1) * SC],
                                start=(sbi == 0),
                                stop=(sbi == nsb - 1),
                            )
                    for q in range(2):
                        # free the psum bank right away; l-pipeline from SBUF
                        ysb = ysbp.tile([65, SC], F32, tag="ysb", name="ysb")
                        if q == 0:
                            nc.scalar.copy(ysb[:, :], psy[q][:, :])
                        else:
                            nc.vector.tensor_copy(ysb[:, :], psy[q][:, :])
                        lraw = lrowp.tile([1, SC], F32, tag="lraw", name="lraw")
                        # partition-shifted copy is HW-safe; a partition-
                        # shifted reciprocal is NOT (garbage on HW, fine in sim)
                        nc.vector.tensor_copy(lraw[0:1, :], ysb[64:65, :])
                        lrow0 = lrowp.tile([1, SC], F32, tag="lrow0", name="lrow0")
                        nc.vector.reciprocal_approx_fast(lrow0[0:1, :], lraw[0:1, :])
                        bc = bcp.tile([64, SC], F32, tag="bc", name="bc")
                        nc.gpsimd.partition_broadcast(bc[:, :], lrow0[0:1, :])
                        nc.vector.tensor_mul(
                            ypair[tci][p][q * 64 : (q + 1) * 64, :],
                            ysb[0:64, :],
                            bc[:, :],
                        )

            def cproj_chunk(tci, pso_pool, ostp, qeng):
                t0 = tci * SC
                for ms in range(4):
                    ost = ostp.tile([128, C], BF16, tag="ost", name="ost")
                    for nch2 in range(2):
                        pso = pso_pool.tile([128, 512], F32, tag="pa", name="pso")
                        for kp in range(2):
                            nc.tensor.matmul(
                                pso[:],
                                ypair[tci][kp][:, bass.ts(ms, 128)],
                                wpT_sb[:, kp, bass.ts(nch2, 512)],
                                start=(kp == 0),
                                stop=(kp == 1),
                            )
                        if nch2 == 0:
                            nc.scalar.copy(ost[:, 0:512], pso[:])
                        else:
                            nc.vector.tensor_copy(ost[:, 512:1024], pso[:])
                    qeng[ms % len(qeng)].dma_start(
                        out=out_d[bass.ds(t0 + ms * 128, 128), :], in_=ost[:]
                    )

            # ---- Single schedule scope: projection, attention, and c_proj
            # share pools so the scheduler can weave them with no stage
            # barrier. PSUM: psA(2) + psE(2x2) + psyE(2) = 8 banks.
            with (
                tc.tile_pool(name="xchp", bufs=3) as xchp,
                tc.tile_pool(name="rawp", bufs=5) as rawp,
                tc.tile_pool(name="ttmp", bufs=3) as ttmp,
                tc.tile_pool(name="ostp", bufs=3) as ostp,
                tc.tile_pool(name="psA", bufs=2, space="PSUM") as psA,
                tc.tile_pool(name="psE", bufs=2, space="PSUM") as psE,
                tc.tile_pool(name="psyE", bufs=1, space="PSUM") as psyE,
            ):

                def proj_chunk(nch, rhs_of, vstat_of):
                    sl = bass.ts(nch, SC)
                    # q,k projection: out[m-tile, t-chunk]
                    raw = [
                        rawp.tile([128, SC], BF16, tag="raw", name=f"raw{m}")
                        for m in range(4)
                    ]
                    for m in range(4):
                        ps = psA.tile([128, SC], F32, tag="pa", name="pa")
                        for ct in range(8):
                            nc.tensor.matmul(
                                ps[:],
                                wqk_sb[:, ct, bass.ts(m, 128)],
                                rhs_of(ct),
                                start=(ct == 0),
                                stop=(ct == 7),
                            )
                        nc.scalar.activation(
                            out=raw[m],
                            in_=ps[:],
                            func=mybir.ActivationFunctionType.Identity,
                            bias=bqk_sb[:, m : m + 1],
                        )
                    # v projection for the 4 t-subtiles of this chunk
                    for tml in range(4):
                        tm = nch * 4 + tml
                        psv = psA.tile([128, 256], F32, tag="pa", name="pav")
                        for ct in range(8):
                            nc.tensor.matmul(
                                psv[:],
                                vstat_of(ct, tml),
                                wv_sb[:, ct, :],
                                start=(ct == 0),
                                stop=(ct == 7),
                            )
                        nc.vector.tensor_add(
                            v_sb[:, tm, :]
                            .rearrange("p (h c) -> p h c", h=HPC)[:, :, 0:64],
                            psv[:].rearrange("p (h c) -> p h c", h=HPC),
                            bv_bc[:].rearrange("p (h c) -> p h c", h=HPC),
                        )
                    # rope on the 4 qk tiles for this chunk
                    for m in range(4):
                        psw = psA.tile([128, SC], F32, tag="pa", name="paw")
                        nc.tensor.matmul(psw[:], psw_sb[:], raw[m][:])
                        tmp = ttmp.tile([128, SC], BF16, tag="ttmp")
                        nc.vector.tensor_mul(tmp[:], psw[:], sin_sb[:, sl])
                        nc.vector.tensor_mul(rot[m][:, sl], raw[m][:], cos_sb[:, sl])
                        nc.vector.tensor_add(rot[m][:, sl], rot[m][:, sl], tmp[:])

                # xch1 races the startup window; xch2/3 and wpT are issued
                # from the gpsimd engine stream mid-loop so their transfers
                # don't steal HBM bandwidth from the startup-critical loads.
                xchs = {1: xchp.tile([128, 8, SC], BF16, tag="xch", name="xch1")}
                nc.sync.dma_start(out=xchs[1][:], in_=xT_r[:, :, bass.ts(1, SC)])
                for nch in range(4):
                    if nch == 0:
                        rhs_of = lambda ct: x0_sb[:, ct, :]
                        vstat_of = lambda ct, tml: x0_sb[:, ct, bass.ts(tml, 128)]
                    else:
                        xch = xchs[nch]
                        rhs_of = lambda ct, xch=xch: xch[:, ct, :]
                        vstat_of = lambda ct, tml, xch=xch: xch[
                            :, ct, bass.ts(tml, 128)
                        ]
                    proj_chunk(nch, rhs_of, vstat_of)
                    # weave in attention as soon as its proj chunk is done
                    # (chunk tci needs proj 0..tci); exp fills ScalarE while
                    # the PE is projection-bound. c_proj of the previous
                    # chunk gives the PE work while exp runs.
                    if nch < 3:
                        attn_chunk(nch, psE, psyE)
                    if nch == 0:
                        # gated loads: pinned after attn0's gpsimd work both
                        # in static order (tile_wait_until) and at runtime
                        # (in-order gpsimd stream)
                        with tc.tile_wait_until(0.02):
                            nc.gpsimd.dma_start(
                                out=cos_sb[:, 1024:2048], in_=cos_d[:, 1024:2048]
                            )
                            nc.gpsimd.dma_start(
                                out=sin_sb[:, 1024:2048], in_=sin_d[:, 1024:2048]
                            )
                            nc.gpsimd.dma_start(
                                out=wpT_sb[:],
                                in_=wpT_d.rearrange("(a b) c -> b a c", b=128),
                            )
                    if nch + 2 <= 3:
                        xchs[nch + 2] = xchp.tile(
                            [128, 8, SC], BF16, tag="xch", name=f"xch{nch + 2}"
                        )
                        with tc.tile_wait_until(0.015 * (nch + 1)):
                            nc.gpsimd.dma_start(
                                out=xchs[nch + 2][:],
                                in_=xT_r[:, :, bass.ts(nch + 2, SC)],
                            )
                    if nch >= 1:
                        cproj_chunk(nch - 1, psA, ostp, [nc.sync])
                attn_chunk(3, psE, psyE)
                cproj_chunk(3, psA, ostp, [nc.sync])

    nc.compile()
    return nc


def _host_shards(x, w_attn, b_attn, w_proj):
    """Per-core input dicts. Core c: batch c//TP, heads [HPC*(c%TP) .. )."""
    pos = np.arange(T, dtype=np.float64)
    div = np.exp(np.arange(0, D, 2, dtype=np.float64) * (-(math.log(10000.0) / D)))
    sinu = np.outer(pos, div)  # [T, 32]
    bf = ml_dtypes.bfloat16
    cosT = np.tile(np.cos(sinu).T, (4, 1)).astype(bf)  # [128, T]
    sinT = np.tile(np.sin(sinu).T, (4, 1)).astype(bf)

    psw = np.zeros((128, 128), dtype=np.float32)  # P[out,in]
    for blk in (0, 64):
        for j in range(32):
            psw[blk + j, blk + 32 + j] = -1.0
            psw[blk + 32 + j, blk + j] = 1.0
    pswapT = np.ascontiguousarray(psw.T).astype(bf)

    ev = np.arange(0, D, 2)
    od = np.arange(1, D, 2)
    in_maps = []
    for c in range(N_CORES):
        b, lane = c // TP, c % TP
        heads = [HPC * lane + i for i in range(HPC)]
        idx_qk = []
        for off in (0, C):  # q rows then k rows, deinterleaved per head
            for p in range(2):
                for hh in (heads[2 * p], heads[2 * p + 1]):
                    base = off + hh * D
                    idx_qk.extend((base + ev).tolist())
                    idx_qk.extend((base + od).tolist())
        idx_qk = np.array(idx_qk)
        idx_v = np.concatenate([2 * C + h * D + np.arange(D) for h in heads])
        cols_p = np.concatenate([h * D + np.arange(D) for h in heads])
        in_maps.append(
            {
                "xT": np.ascontiguousarray(x[b].T).astype(bf),
                "wqk": np.ascontiguousarray(w_attn[idx_qk, :].T).astype(bf),
                "wv": np.ascontiguousarray(w_attn[idx_v, :].T).astype(bf),
                "wpT": np.ascontiguousarray(w_proj[:, cols_p].T).astype(bf),
                "bqk": np.ascontiguousarray(b_attn[idx_qk].reshape(4, 128)),
                "bv": np.ascontiguousarray(b_attn[idx_v].reshape(1, 256)),
                "cosT": cosT,
                "sinT": sinT,
                "pswapT": pswapT,
            }
        )
    return in_maps


def kernel(x, w_attn, b_attn, w_proj, b_proj, _trace=False):
    x = np.asarray(x, dtype=np.float32)
    w_attn = np.asarray(w_attn, dtype=np.float32)
    b_attn = np.asarray(b_attn, dtype=np.float32)
    w_proj = np.asarray(w_proj, dtype=np.float32)
    b_proj = np.asarray(b_proj, dtype=np.float32)

    if "nc" not in _cached:
        _cached["nc"] = _build_program()
    nc = _cached["nc"]

    in_maps = _host_shards(x, w_attn, b_attn, w_proj)
    res = bass_utils.run_bass_kernel_spmd(
        nc, in_maps, core_ids=list(range(N_CORES)), trace=_trace
    )
    _cached["last_result"] = res

    out = np.empty((B, T, C), dtype=np.float32)
    for b in range(B):
        acc = res.results[b * TP]["out"].astype(np.float32)
        for lane in range(1, TP):
            acc = acc + res.results[b * TP + lane]["out"].astype(np.float32)
        out[b] = acc + b_proj[None, :]
    return out
